# revision 11
# baseline (speedup 1.0000x reference)
"""Trainium2 Bass kernel for cache-augmented attention.

Reference computation (per full input):
    q = (x @ Wq.T + bq) / sqrt(hd), split into 8 heads of 96
    scores[b,h,s,n] = q_h[s] . ck_h[n] - 0.1*age[n]
    attn = softmax(scores over n);  ctx = attn @ cv_h
    out = (x + ctx @ Wo.T + bo - mu)/sigma * g + b   (layernorm)

Sharding: data-parallel over the 8192 = B*S token rows, 1024 rows per
core; cache bank + weights replicated.  No collectives.

Per-core design:
  - Everything runs "transposed" (feature dim on partitions, tokens on
    the free axis) so softmax reductions contract over the cache axis
    on the PE (no cross-partition reductions anywhere).
  - age penalty folded multiplicatively: p = exp(scores),
    ctx_aug = p.T @ [w*cv | w] with w = exp(-0.1*age); row 96 of the
    ctx accumulator is the softmax denominator for free.
  - heads (96 wide) zero-padded to 128 so every transpose can use the
    DMA xbar (dma_start_transpose needs free%128==0) and matmul
    contractions use full 128 partitions.
  - bf16 matmul operands; fp32 residual + layernorm.
  - SBUF slots of phase-dead tensors (xT, qT, ckT) are re-used by
    later phases via tile-pool tags.
"""

import threading

import ml_dtypes
import numpy as np

import concourse.bass as bass
import concourse.mybir as mybir
import concourse.tile as tile
from concourse.bass_utils import run_bass_kernel_spmd

B, S, H, N, NH = 2, 4096, 768, 2048, 8
HD = H // NH          # 96
NCORES = 8
R = (B * S) // NCORES  # 1024 rows per core
SW = R                # free-axis width for the main phase (1024)
NC2 = N // 128        # 16 cache chunks of 128
KC = H // 128         # 6 chunks of the hidden dim
ST = R // 128         # 8 token tiles per core
SCALE = 1.0 / float(np.sqrt(HD))

F32 = mybir.dt.float32
BF16 = mybir.dt.bfloat16
AF = mybir.ActivationFunctionType
ALU = mybir.AluOpType



# ---------------------------------------------------------------------------
# BIR legalizer: this container's walrus accepts at most ONE sync wait (and
# one sync update) per instruction, while Tile emits multi-wait instructions.
# Hoist extra waits onto same-engine Drain nops inserted just before the
# instruction (sem waits commute; streams execute in order => semantics
# preserved).  Extra updates ride on Drains just after.
import json as _json

_MAX_WAITS = 1
_MAX_UPDATES = 1


def _mk_drain(name, engine, waits, updates, debug):
    return {
        "debug": debug,
        "engine": engine,
        "ins": [],
        "name": name,
        "opcode": "Drain",
        "outs": [],
        "sync_info": {"on_wait": waits, "on_update": updates},
    }


def _legalize_block(block, counter):
    out = []
    for inst in block.get("instructions", []):
        si = inst.get("sync_info")
        waits = list(si.get("on_wait") or []) if si else []
        updates = list(si.get("on_update") or []) if si else []
        eng = inst.get("engine")
        pre, post = [], []
        if len(waits) > _MAX_WAITS and eng not in (None, "Unassigned"):
            extra, keep = waits[:-_MAX_WAITS], waits[-_MAX_WAITS:]
            for w in extra:
                counter[0] += 1
                pre.append(_mk_drain(f"LGW-{counter[0]}", eng, [w], [],
                                     inst.get("debug")))
            si["on_wait"] = keep
        if len(updates) > _MAX_UPDATES and eng not in (None, "Unassigned"):
            keep, extra = updates[:_MAX_UPDATES], updates[_MAX_UPDATES:]
            for u in extra:
                counter[0] += 1
                post.append(_mk_drain(f"LGU-{counter[0]}", eng, [], [u],
                                      inst.get("debug")))
            si["on_update"] = keep
        out.extend(pre)
        out.append(inst)
        out.extend(post)
    block["instructions"] = out
    for sub in block.get("blocks", []) or []:
        _legalize_block(sub, counter)


def _legalize_bir_json(data):
    m = _json.loads(data)
    counter = [0]
    for f in m.get("functions", []):
        for b in f.get("blocks", []) or []:
            _legalize_block(b, counter)
    return _json.dumps(m).encode()


def _install_legalizer(nc):
    if getattr(nc, "_birlegal_installed", False):
        return nc
    orig = nc.to_json_bytes
    nc.to_json_bytes = lambda: _legalize_bir_json(orig())
    nc._birlegal_installed = True
    return nc


def _build_program():
    nc = bass.Bass(name="cache_attn")

    x_h = nc.dram_tensor("xs", [R, H], F32, kind="ExternalInput")
    wq_h = nc.dram_tensor("Wq", [H, H], F32, kind="ExternalInput")
    bq_h = nc.dram_tensor("bq", [H], F32, kind="ExternalInput")
    wo_h = nc.dram_tensor("Wo", [H, H], F32, kind="ExternalInput")
    bo_h = nc.dram_tensor("bo", [H], F32, kind="ExternalInput")
    ck_h = nc.dram_tensor("cache_keys", [N, H], F32, kind="ExternalInput")
    cv_h = nc.dram_tensor("cache_values", [N, H], F32, kind="ExternalInput")
    age_h = nc.dram_tensor("cache_age", [N], F32, kind="ExternalInput")
    g_h = nc.dram_tensor("ln_g", [H], F32, kind="ExternalInput")
    b_h = nc.dram_tensor("ln_b", [H], F32, kind="ExternalInput")
    selh_h = nc.dram_tensor("selh", [NH, NH * HD], BF16, kind="ExternalInput")
    out_h = nc.dram_tensor("out", [R, H], F32, kind="ExternalOutput")

    # HBM scratch for repacking softmax denominators across partitions.
    den_d = nc.dram_tensor("den_scratch", [NH, SW], BF16)
    rden_d = nc.dram_tensor("rden_scratch", [NH, SW], BF16)

    with tile.TileContext(nc) as tc:
        with (
            tc.tile_pool(name="const", bufs=1) as const,
            tc.tile_pool(name="persist", bufs=1) as big,
            tc.tile_pool(name="wload", bufs=2) as wload,
            tc.tile_pool(name="padbuf", bufs=2) as padbuf,
            tc.tile_pool(name="pwork", bufs=3) as pwork,
            tc.tile_pool(name="small", bufs=16) as small,
        ):
            # ---------------- constants / small tensors ----------------
            age_sb = const.tile([128, NC2], F32, tag="age", name="age")
            nc.sync.dma_start(age_sb, age_h[:].rearrange("(c p) -> p c", p=128))
            w_sb = const.tile([128, NC2], F32, tag="w", name="w")
            nc.scalar.activation(w_sb, age_sb, AF.Exp, scale=-0.1)
            ones8 = const.tile([128, NH], F32, tag="ones8", name="ones8")
            nc.vector.memset(ones8, 1.0)

            bq_sb = const.tile([HD, NH], F32, tag="bq", name="bq")
            nc.sync.dma_start(bq_sb, bq_h[:].rearrange("(h p) -> p h", p=HD))
            bqs_sb = const.tile([HD, NH], F32, tag="bqs", name="bqs")
            nc.scalar.mul(bqs_sb, bq_sb, SCALE)
            bo_sb = const.tile([128, KC], F32, tag="bo", name="bo")
            nc.sync.dma_start(bo_sb, bo_h[:].rearrange("(m p) -> p m", p=128))

            def _bcast128(ap):
                return bass.AP(tensor=ap.tensor, offset=ap.offset,
                               ap=[[0, 128]] + list(ap.ap))

            g_sb = const.tile([128, H], F32, tag="g", name="g")
            nc.sync.dma_start(g_sb, _bcast128(g_h[:]))
            b_sb = const.tile([128, H], F32, tag="b", name="b")
            nc.sync.dma_start(b_sb, _bcast128(b_h[:]))
            eps_sb = const.tile([128, 1], F32, tag="eps", name="eps")
            nc.vector.memset(eps_sb, 1e-5)
            # one-hot head selectors for broadcasting the denominator rows:
            # sel[:, 96h:96h+96] has row h = 1, rest 0 (supplied from host:
            # engine writes at non-32-aligned partition offsets are illegal)
            sel = const.tile([NH, NH * HD], BF16, tag="sel", name="sel")
            nc.sync.dma_start(sel, selh_h[:])

            # ---------------- weights: load, scale, transpose ----------
            # WqT[kc] : [128(hc), 768(ho)] bf16, scaled by 1/sqrt(hd)
            wqT = [big.tile([128, H], BF16, tag=f"wqT{kc}", name=f"wqT{kc}")
                   for kc in range(KC)]
            wq_bf = []
            for mo in range(KC):
                wt = wload.tile([128, H], F32, tag="wload", name="wload")
                nc.sync.dma_start(wt, wq_h[128 * mo:128 * (mo + 1), :])
                wb = padbuf.tile([128, H], BF16, tag="xbf", name="xbf")
                nc.scalar.mul(wb, wt, SCALE)
                wq_bf.append(wb)
            for mo in range(KC):
                for kc in range(KC):
                    nc.sync.dma_start_transpose(
                        wqT[kc][:, 128 * mo:128 * (mo + 1)],
                        wq_bf[mo][:, 128 * kc:128 * (kc + 1)],
                    )

            # WoT[h] : [128(hc pad), 768(ho)] bf16, rows 96:128 zero
            woT = [big.tile([128, H], BF16, tag=f"woT{h}", name=f"woT{h}")
                   for h in range(NH)]
            for mo in range(KC):
                wt = wload.tile([128, H], F32, tag="wload", name="wload")
                nc.sync.dma_start(wt, wo_h[128 * mo:128 * (mo + 1), :])
                wp = padbuf.tile([128, NH * 128], BF16, tag="padb", name="padb")
                nc.vector.memset(wp, 0.0)
                nc.vector.tensor_copy(
                    wp[:].rearrange("p (h c) -> p h c", c=128)[:, :, 0:HD],
                    wt[:].rearrange("p (h c) -> p h c", c=HD),
                )
                for h in range(NH):
                    nc.sync.dma_start_transpose(
                        woT[h][:, 128 * mo:128 * (mo + 1)],
                        wp[:, 128 * h:128 * (h + 1)],
                    )

            # ckT[h] : [128(hd pad), 2048(n)] bf16, rows 96:128 zero
            ckT = [big.tile([128, N], BF16, tag=f"ckT{h}", name=f"ckT{h}")
                   for h in range(NH)]
            for c in range(NC2):
                ct = wload.tile([128, H], F32, tag="wload", name="wload")
                nc.sync.dma_start(ct, ck_h[128 * c:128 * (c + 1), :])
                cp = padbuf.tile([128, NH * 128], BF16, tag="padb", name="padb")
                nc.vector.memset(cp, 0.0)
                nc.vector.tensor_copy(
                    cp[:].rearrange("p (h c) -> p h c", c=128)[:, :, 0:HD],
                    ct[:].rearrange("p (h c) -> p h c", c=HD),
                )
                for h in range(NH):
                    nc.sync.dma_start_transpose(
                        ckT[h][:, 128 * c:128 * (c + 1)],
                        cp[:, 128 * h:128 * (h + 1)],
                    )

            # cvw[c] : [128(n), 8*97] bf16 -- per head: 96 cols of w*cv,
            # then one col of w (softmax denominator accumulator).
            cvw = [big.tile([128, NH * (HD + 1)], BF16, tag=f"cvw{c}",
                            name=f"cvw{c}") for c in range(NC2)]
            for c in range(NC2):
                ct = wload.tile([128, H], F32, tag="wload", name="wload")
                nc.sync.dma_start(ct, cv_h[128 * c:128 * (c + 1), :])
                cw = cvw[c]
                v3 = cw[:].rearrange("p (h c) -> p h c", c=HD + 1)
                nc.scalar.mul(
                    v3[:, :, 0:HD],
                    ct[:].rearrange("p (h c) -> p h c", c=HD),
                    w_sb[:, c:c + 1],
                )
                nc.scalar.mul(
                    v3[:, :, HD:HD + 1].rearrange("p h c -> p (h c)"),
                    ones8, w_sb[:, c:c + 1],
                )

            # ---------------- x: load + transpose --------------------
            xT = [big.tile([128, SW], BF16, tag=f"xT{kc}", name=f"xT{kc}")
                  for kc in range(KC)]
            for st in range(ST):
                xt = wload.tile([128, H], F32, tag="wload", name="wload")
                nc.sync.dma_start(xt, x_h[128 * st:128 * (st + 1), :])
                xb = padbuf.tile([128, H], BF16, tag="xbf", name="xbf")
                nc.vector.tensor_copy(xb, xt)
                for kc in range(KC):
                    nc.sync.dma_start_transpose(
                        xT[kc][:, 128 * st:128 * (st + 1)],
                        xb[:, 128 * kc:128 * (kc + 1)],
                    )

            # ---------------- phase A: q projection -------------------
            qT = [big.tile([128, SW], BF16, tag=f"qT{h}", name=f"qT{h}")
                  for h in range(NH)]
            with tc.tile_pool(name="pq", bufs=2, space="PSUM") as pq:
                for h in range(NH):
                    qp = pq.tile([HD, SW], F32, tag="qp", name="qp")
                    for kc in range(KC):
                        lw = wqT[kc][:, HD * h:HD * (h + 1)]
                        for j in range(2):
                            nc.tensor.matmul(
                                qp[:, 512 * j:512 * (j + 1)],
                                lw,
                                xT[kc][:, 512 * j:512 * (j + 1)],
                                start=(kc == 0), stop=(kc == KC - 1),
                            )
                    nc.vector.memset(qT[h][HD:128, :], 0.0)
                    nc.scalar.add(qT[h][0:HD, :], qp, bqs_sb[:, h:h + 1])

            # ---------------- phase B: attention ----------------------
            ctxc = [big.tile([HD + 1, SW], BF16, tag=f"ctxc{h}",
                             name=f"ctxc{h}") for h in range(NH)]
            with (
                tc.tile_pool(name="psc", bufs=3, space="PSUM") as psc,
                tc.tile_pool(name="pctx", bufs=1, space="PSUM") as pctx,
            ):
                for h in range(NH):
                    ctxp = pctx.tile([HD + 1, SW], F32, tag="ctx", name="ctx")
                    for c in range(NC2):
                        sc = psc.tile([128, SW], F32, tag="sc", name="sc")
                        for j in range(2):
                            nc.tensor.matmul(
                                sc[:, 512 * j:512 * (j + 1)],
                                ckT[h][:, 128 * c:128 * (c + 1)],
                                qT[h][:, 512 * j:512 * (j + 1)],
                                start=True, stop=True,
                            )
                        p = pwork.tile([128, SW], BF16, tag="p", name="p")
                        nc.scalar.activation(p, sc, AF.Exp)
                        lw = cvw[c][:, (HD + 1) * h:(HD + 1) * (h + 1)]
                        for j in range(2):
                            nc.tensor.matmul(
                                ctxp[:, 512 * j:512 * (j + 1)],
                                lw,
                                p[:, 512 * j:512 * (j + 1)],
                                start=(c == 0), stop=(c == NC2 - 1),
                            )
                    nc.vector.tensor_copy(ctxc[h], ctxp)
                    # denominator row -> HBM scratch for cross-partition repack
                    nc.sync.dma_start(den_d[h, :], ctxc[h][HD:HD + 1, :])

            # reciprocal of denominators, batched across 128 partitions
            dpack = const.tile([128, SW // 16], BF16, tag="dpack", name="dpack")
            nc.sync.dma_start(
                dpack, den_d[:, :].rearrange("h (a b) -> (h a) b", b=SW // 16))
            rdenf = const.tile([128, SW // 16], F32, tag="rdenf", name="rdenf")
            nc.vector.reciprocal(rdenf, dpack)
            rdenb = const.tile([128, SW // 16], BF16, tag="rdenb", name="rdenb")
            nc.vector.tensor_copy(rdenb, rdenf)
            nc.sync.dma_start(
                rden_d[:, :].rearrange("h (a b) -> (h a) b", b=SW // 16), rdenb)
            rd8 = const.tile([NH, SW], BF16, tag="rd8", name="rd8")
            nc.sync.dma_start(rd8, rden_d[:, :])

            # ---------------- phase B2: normalize ctx -----------------
            # ctxn[h] reuses the (dead) xT slots for h<6
            ctxn = [big.tile([128, SW], BF16,
                             tag=(f"xT{h}" if h < KC else f"ctxn{h}"),
                             name=f"ctxn{h}") for h in range(NH)]
            with tc.tile_pool(name="pbc", bufs=2, space="PSUM") as pbc:
                for h in range(NH):
                    bc = pbc.tile([HD, SW], F32, tag="bc", name="bc")
                    for j in range(2):
                        nc.tensor.matmul(
                            bc[:, 512 * j:512 * (j + 1)],
                            sel[:, HD * h:HD * (h + 1)],
                            rd8[:, 512 * j:512 * (j + 1)],
                            start=True, stop=True,
                        )
                    nc.vector.memset(ctxn[h][HD:128, :], 0.0)
                    nc.vector.tensor_mul(ctxn[h][0:HD, :], ctxc[h][0:HD, :], bc)

            # ---------------- phase C: out projection -----------------
            # outc[mo] reuses the (dead) qT slots
            outc = [big.tile([128, SW], BF16, tag=f"qT{mo}", name=f"outc{mo}")
                    for mo in range(KC)]
            with tc.tile_pool(name="pop", bufs=2, space="PSUM") as pop:
                for mo in range(KC):
                    op = pop.tile([128, SW], F32, tag="op", name="op")
                    for h in range(NH):
                        lw = woT[h][:, 128 * mo:128 * (mo + 1)]
                        for j in range(2):
                            nc.tensor.matmul(
                                op[:, 512 * j:512 * (j + 1)],
                                lw,
                                ctxn[h][:, 512 * j:512 * (j + 1)],
                                start=(h == 0), stop=(h == NH - 1),
                            )
                    nc.scalar.add(outc[mo], op, bo_sb[:, mo:mo + 1])

            # -------- phase D: transpose back, residual, layernorm ----
            # all the [128, 768-ish] work tiles ride on dead ckT slots
            for st in range(ST):
                nat = big.tile([128, H], BF16, tag="ckT5", name="nat")
                for mo in range(KC):
                    nc.sync.dma_start_transpose(
                        nat[:, 128 * mo:128 * (mo + 1)],
                        outc[mo][:, 128 * st:128 * (st + 1)],
                    )
                xd = big.tile([128, H], F32, tag="ckT6", name="xd")
                nc.sync.dma_start(xd, x_h[128 * st:128 * (st + 1), :])
                y = big.tile([128, H], F32, tag="ckT0", name="y")
                nc.vector.tensor_add(y, nat, xd)
                # mean/var in one pass (3 subgroups of 256)
                stats = small.tile(
                    [128, 3, nc.vector.BN_STATS_DIM], F32,
                    tag="stats", name="stats")
                yv = y[:].rearrange("p (a b) -> p a b", b=256)
                for sg in range(3):
                    nc.vector.bn_stats(stats[:, sg, :], yv[:, sg, :])
                mv = small.tile(
                    [128, nc.vector.BN_AGGR_DIM], F32, tag="mv", name="mv")
                nc.vector.bn_aggr(mv, stats)
                mu_neg = small.tile([128, 1], F32, tag="mu", name="mu_neg")
                nc.scalar.mul(mu_neg, mv[:, 0:1], -1.0)
                yc = big.tile([128, H], F32, tag="ckT1", name="yc")
                nc.scalar.add(yc, y, mu_neg)
                std = small.tile([128, 1], F32, tag="std", name="std")
                nc.scalar.activation(std, mv[:, 1:2], AF.Sqrt, bias=eps_sb)
                rstd = small.tile([128, 1], F32, tag="rstd", name="rstd")
                nc.vector.reciprocal(rstd, std)
                t1 = big.tile([128, H], F32, tag="ckT3", name="t1")
                nc.scalar.mul(t1, yc, rstd)
                t2 = big.tile([128, H], F32, tag="ckT7", name="t2")
                nc.vector.tensor_mul(t2, t1, g_sb)
                outf = big.tile([128, H], F32, tag="ckT4", name="outf")
                nc.gpsimd.tensor_add(outf, t2, b_sb)
                nc.sync.dma_start(out_h[128 * st:128 * (st + 1), :], outf)

    return _install_legalizer(nc)


_lock = threading.Lock()
_cached = {}


def _get_program():
    with _lock:
        if "nc" not in _cached:
            _cached["nc"] = _build_program()
        return _cached["nc"]


def kernel(**inputs):
    inputs = {k: np.ascontiguousarray(np.asarray(v, dtype=np.float32))
              for k, v in inputs.items()}
    x = inputs["inputs"].reshape(B * S, H)

    sel = np.zeros((NH, NH * HD), dtype=ml_dtypes.bfloat16)
    for h in range(NH):
        sel[h, HD * h:HD * (h + 1)] = 1.0

    nc = _get_program()
    in_maps = []
    for i in range(NCORES):
        in_maps.append({
            "xs": np.ascontiguousarray(x[R * i:R * (i + 1)]),
            "selh": sel,
            "Wq": inputs["Wq"],
            "bq": inputs["bq"],
            "Wo": inputs["Wo"],
            "bo": inputs["bo"],
            "cache_keys": inputs["cache_keys"],
            "cache_values": inputs["cache_values"],
            "cache_age": inputs["cache_age"],
            "ln_g": inputs["ln_g"],
            "ln_b": inputs["ln_b"],
        })

    res = run_bass_kernel_spmd(nc, in_maps, list(range(NCORES)))
    out = np.concatenate([res.results[i]["out"] for i in range(NCORES)], axis=0)
    return out.reshape(B, S, H).astype(np.float32)
